# revision 1
# baseline (speedup 1.0000x reference)
"""Trainium2 Bass kernel for causal multi-head attention block.

Module: qkv = x @ W_attn + b_attn; causal softmax((q k^T)/sqrt(C)); y = (attn @ v) @ W_proj + b_proj
Shapes (hardcoded): x [8, 1024, 768], W_attn [768, 2304], W_proj [768, 768], H=12, D=64.

Sharding: data parallel over batch - core b computes batch element b (B == n_cores == 8).
No collectives; host scatters x and gathers y. Host pre-casts x/W to bf16.

Per-core dataflow (all matmul operands bf16, fp32 PSUM accumulate):
  1. x arrives host-pre-transposed as x^T [768,1024] bf16; 6 plain loads
  2. QT,KT [c,1024] = (W_attn chunk).T @ xT   (W stationary, xT moving)
     V natural [tok, 12*(64+1)]: per head [V_h | 1.0] so the AV matmul emits
     the softmax denominator as the 65th output column.
  3. per head h, blocks (query-group g, key-tile j) packed first-fit-decreasing
     into exactly-full 512-word PSUM bins, one bank per scores tile (sc x3 /
     av x3 ring depths beat wider tiles with fewer exps):
       scoresT[tk,tq] = KT_h.T @ QT_h   (causal-tight tiles only)
       e = Exp(scoresT/sqrt(768))       (ScalarE, PSUM->SBUF bf16)
       diag tiles *= upper-tri mask     (Pool)
       avT[tq, 65] += e_block.T @ [V_h|1]  (65 moving rows/tile-pair: ~2x
                                            fewer PE rows than 512-wide AV)
       Y[t, h*64:] = avT[:,0:64] * recip(avT[:,64])  (normalization fused
                                            into the PSUM->SBUF copy, DVE)
     Scores run two heads ahead of AV; QK-chunk matmuls interleave as PE
     filler so exp latency and fin pacing never stall the PE.
  4. Y -> yT via PE transposes; out[tq, c] = yT.T @ W_proj (+bias), DMA out.
"""

import math
from collections import deque

import numpy as np

import concourse.bass as bass
import concourse.mybir as mybir
from concourse import bacc, tile
from concourse.bass_utils import run_bass_kernel_spmd
from concourse.masks import make_identity, make_upper_triangular

N_CORES = 8
B, T, C = 8, 1024, 768
H, D = 12, 64
C3 = 3 * C
NT = T // 128          # 8 token tiles
NK = C // 128          # 6 contraction chunks
NG = T // 512          # 2 query groups of 512
SCALE = 1.0 / math.sqrt(C)

F32 = mybir.dt.float32
BF16 = mybir.dt.bfloat16
MULT = mybir.AluOpType.mult
EXP = mybir.ActivationFunctionType.Exp


def _head_plan():
    """Causal score blocks for one head, packed into PSUM tiles.

    Returns (tiles, ecol): tiles is a list of (blocks, tile_width) where each
    block is (g, j, nw, tq0, diag, sc_off, e_off); ecol[(t, j)] gives the
    column in the head's e tile holding scoresT[:, tq-tile t] for key-tile j.
    """
    blocks = []
    for g in range(NG):
        for j in range(4 * g + 4):
            nw = 512 - max(j * 128 - g * 512, 0)
            tq0 = g * 512 + (512 - nw)
            blocks.append([g, j, nw, tq0])
    # first-fit-decreasing into 512-wide bins (bins end up exactly full)
    blocks.sort(key=lambda b: -b[2])
    bins = []
    for b in blocks:
        for bn in bins:
            if sum(x[2] for x in bn) + b[2] <= 512:
                bn.append(b)
                break
        else:
            bins.append([b])
    # one bin per [128,512] PSUM tile; assign sc/e offsets
    tiles = []
    e_base = 0
    for bn in bins:
        tb, off = [], 0
        for g, j, nw, tq0 in bn:
            diag = j * 128 == tq0
            tb.append((g, j, nw, tq0, diag, off, e_base + off))
            off += nw
        tiles.append((tb, off))
        e_base += off
    assert e_base == 4608
    ecol = {}
    for tb, _ in tiles:
        for g, j, nw, tq0, diag, sc_off, e_off in tb:
            for t in range(4 * g, 4 * g + 4):
                lo = t * 128
                if tq0 <= lo and lo + 128 <= tq0 + nw:
                    ecol[(t, j)] = e_off + lo - tq0
    for t in range(NT):
        for j in range(t + 1):
            assert (t, j) in ecol, (t, j)
    return tiles, ecol


_PLAN_TILES, _PLAN_ECOL = _head_plan()


def _build_nc(has_battn: bool, has_bproj: bool, num_devices: int, repeat: int = 1):
    nc = bacc.Bacc(
        "TRN2", target_bir_lowering=False, debug=False, num_devices=num_devices
    )

    x_d = nc.dram_tensor("x", (C, T), BF16, kind="ExternalInput")  # pre-transposed on host
    wa_d = nc.dram_tensor("W_attn", (C, C3), BF16, kind="ExternalInput")
    wp_d = nc.dram_tensor("W_proj", (C, C), BF16, kind="ExternalInput")
    ba_d = bp_d = None
    if has_battn:
        ba_d = nc.dram_tensor("b_attn", (C3,), F32, kind="ExternalInput")
    if has_bproj:
        bp_d = nc.dram_tensor("b_proj", (C,), F32, kind="ExternalInput")
    y_d = nc.dram_tensor("y", (T, C), F32, kind="ExternalOutput")

    with tile.TileContext(nc) as tc:
        with (
            tc.tile_pool(name="sb", bufs=1) as sb,
            tc.tile_pool(name="ps", bufs=1, space="PSUM") as ps,
        ):
            for r in range(repeat):
                _emit(nc, sb, ps, x_d, wa_d, wp_d, ba_d, bp_d, y_d,
                      warmup=(r == 0))
    nc.compile()
    return nc


def _emit(nc, sb, ps, x_d, wa_d, wp_d, ba_d, bp_d, y_d, warmup=True):
    def st(shape, tag, dtype=BF16, name=None):
        return sb.tile(list(shape), dtype, tag=tag, bufs=_SB_BUFS[tag],
                       name=name or tag)

    def pt(shape, tag, dtype=F32, name=None):
        return ps.tile(list(shape), dtype, tag=tag, bufs=_PS_BUFS[tag],
                       name=name or tag)

    # --- constants ---
    ident32 = st([128, 128], "ident32", dtype=F32)
    make_identity(nc, ident32[:])
    ident = st([128, 128], "ident")
    nc.vector.tensor_copy(ident[:], ident32[:])
    tri32 = st([128, 128], "tri32", dtype=F32)
    make_upper_triangular(nc, tri32[:], val=1.0, diag=True)  # tri[r,c]=1 iff c>=r
    tri = st([128, 128], "tri")
    nc.vector.tensor_copy(tri[:], tri32[:])

    bqk = []
    ones = bv_row = bp_row = None
    if ba_d is not None or bp_d is not None:
        ones32 = st([1, 128], "ones32", dtype=F32)
        nc.gpsimd.memset(ones32[:], 1.0)
        ones = st([1, 128], "ones")
        nc.vector.tensor_copy(ones[:], ones32[:])
    if ba_d is not None:
        for m in range(12):  # Q,K outchan tiles 0..1535
            bt = st([128, 1], "bqk", dtype=F32)
            nc.sync.dma_start(bt[:], ba_d.ap()[m * 128 : (m + 1) * 128])
            bqk.append(bt)
        bv32 = st([1, C], "bv32", dtype=F32)
        nc.sync.dma_start(bv32[:], ba_d.ap()[2 * C : 3 * C])
        bv_row = st([1, C], "bvrow")
        nc.vector.tensor_copy(bv_row[:], bv32[:])
    if bp_d is not None:
        bp32 = st([1, C], "bp32", dtype=F32)
        nc.sync.dma_start(bp32[:], bp_d.ap())
        bp_row = st([1, C], "bprow")
        nc.vector.tensor_copy(bp_row[:], bp32[:])

    # --- persistent activations ---
    qT = [st([128, T], "qkT", name=f"qT{c}") for c in range(NK)]
    kT = [st([128, T], "qkT", name=f"kT{c}") for c in range(NK)]
    vt = [st([128, H * 65], "v", name=f"v{t}") for t in range(NT)]
    Yn = [st([128, C], "ynat", name=f"Y{t}") for t in range(NT)]
    yTc = [st([128, T], "yT", name=f"yT{c}") for c in range(NK)]

    # --- DMAs (SP queue; emission order = issue order) ---
    was = []

    def load_wa(m):
        wam = st([128, NK * 128], "wa", name=f"wa{m}")
        nc.sync.dma_start(
            wam[:],
            wa_d.ap()[:, m * 128 : (m + 1) * 128].rearrange(
                "(k p) n -> p k n", p=128
            ),
        )
        return wam

    was.append(load_wa(0))      # Q chunk 0
    was.append(load_wa(6))      # K chunk 0
    xT = []
    for c in range(NK):
        xt = st([128, T], "xT", name=f"xT{c}")
        xT.append(xt)
        nc.sync.dma_start(xt[:], x_d.ap()[c * 128 : (c + 1) * 128, :])
    was.append(load_wa(1))
    was.append(load_wa(7))
    # all W_attn V columns, every k-chunk, in one 3-D-AP DMA
    wvall = st([128, NK * C], "wv")
    nc.sync.dma_start(
        wvall[:],
        wa_d.ap()[:, 2 * C : 3 * C].rearrange("(k p) n -> p k n", p=128),
    )
    for c in range(2, NK):
        was.append(load_wa(c))
        was.append(load_wa(c + 6))
    wa_of = {}
    order = [0, 6, 1, 7] + [m for c in range(2, NK) for m in (c, c + 6)]
    for i, m in enumerate(order):
        wa_of[m] = was[i]
    wpall = st([128, NK * C], "wp")
    nc.sync.dma_start(
        wpall[:], wp_d.ap().rearrange("(k p) n -> p k n", p=128)
    )

    # --- QK-chunk unit: thunk list (filler granularity = 1 matmul) ---
    def qk_unit_thunks(dst, m, cp_eng=None):
        ths = []
        for g in range(NG):
            box = {}

            def mk(k, g=g, box=box):
                def run():
                    if k == 0:
                        box["acc"] = pt([128, 512], "qkv")
                    nc.tensor.matmul(
                        box["acc"][:],
                        wa_of[m][:, k * 128 : (k + 1) * 128],
                        xT[k][:, g * 512 : (g + 1) * 512],
                        start=(k == 0),
                        stop=(k == NK - 1),
                    )
                return run

            ths += [mk(k) for k in range(NK)]

            def cp(g=g, box=box, m=m):
                dst_ap = dst[:, g * 512 : (g + 1) * 512]
                if ba_d is not None:
                    nc.vector.tensor_scalar_add(dst_ap, box["acc"][:], bqk[m][:])
                elif cp_eng == "act":
                    nc.scalar.copy(dst_ap, box["acc"][:])
                else:
                    nc.vector.tensor_copy(dst_ap, box["acc"][:])

            ths.append(cp)
        return ths

    # --- V natural [tok, 12*(64+1)] ---
    def emit_v(trange):
        for t in trange:
            nc.gpsimd.memset(vt[t][:], 1.0)  # ones cols; data cols overwritten
            for co, w in ((0, 512), (512, 256)):
                acc = pt([128, w], "qkv")
                for k in range(NK):
                    nc.tensor.matmul(
                        acc[:],
                        xT[k][:, t * 128 : (t + 1) * 128],
                        wvall[:, k * C + co : k * C + co + w],
                        start=(k == 0),
                        stop=(k == NK - 1) and ba_d is None,
                    )
                if ba_d is not None:
                    nc.tensor.matmul(
                        acc[:],
                        ones[0:1, 0:128],
                        bv_row[0:1, co : co + w],
                        start=False,
                        stop=True,
                    )
                nc.vector.tensor_copy(
                    vt[t][:].rearrange("p (h e) -> p h e", e=65)[
                        :, co // D : (co + w) // D, 0:D
                    ],
                    acc[:].rearrange("p (h e) -> p h e", e=D),
                )

    # --- per-head scores + exp + mask ---
    e_of = {}

    def emit_sc(h):
        ch, off = h // 2, (h % 2) * 64
        e = st([128, 4608], "expt", name=f"e{h}")
        e_of[h] = e
        for tb, tw in _PLAN_TILES:
            sc = pt([128, 512], "sc")
            for g, j, nw, tq0, diag, sc_off, e_off in tb:
                nc.tensor.matmul(
                    sc[:, sc_off : sc_off + nw],
                    kT[ch][off : off + 64, j * 128 : (j + 1) * 128],
                    qT[ch][off : off + 64, tq0 : tq0 + nw],
                    start=True,
                    stop=True,
                )
            e_base = tb[0][6]
            nc.scalar.activation(
                e[:, e_base : e_base + tw], sc[:, 0:tw], EXP, scale=SCALE
            )
            for g, j, nw, tq0, diag, sc_off, e_off in tb:
                if diag:
                    nc.gpsimd.tensor_tensor(
                        e[:, e_off : e_off + 128],
                        e[:, e_off : e_off + 128],
                        tri[:],
                        op=MULT,
                    )

    # --- per-head AV^T + fused-normalization fin, interleaved with filler ---
    def tp_pe(chh, t, tag="qkv"):
        ptt = pt([128, 128], tag, dtype=BF16)
        nc.tensor.transpose(
            ptt[:], Yn[t][:, chh * 128 : (chh + 1) * 128], ident[:]
        )
        nc.vector.tensor_copy(yTc[chh][:, t * 128 : (t + 1) * 128], ptt[:])

    def _avT_tile(h, t, e):
        av = pt([128, 65], "av")
        for j in range(t + 1):
            nc.tensor.matmul(
                av[:],
                e[:, _PLAN_ECOL[(t, j)] : _PLAN_ECOL[(t, j)] + 128],
                vt[j][:, h * 65 : (h + 1) * 65],
                start=(j == 0),
                stop=(j == t),
            )
        rc = st([128, 1], "rc", dtype=F32)
        nc.vector.reciprocal(rc[:], av[:, 64:65])
        nc.vector.tensor_scalar_mul(
            Yn[t][:, h * D : (h + 1) * D], av[:, 0:D], rc[:]
        )

    def emit_avT(h, filler, inline_tp=None, cb=None):
        e = e_of.pop(h)
        for t in range(NT):
            _avT_tile(h, t, e)
            if inline_tp is not None:
                tp_pe(inline_tp, t)
            if cb is not None:
                cb(t)
            filler.step(2 if t < 5 else 1)

    def emit_avT_pair(ha, hb, filler, inline_tp=(None, None)):
        ea, eb = e_of.pop(ha), e_of.pop(hb)
        for t in range(NT):
            _avT_tile(ha, t, ea)
            if inline_tp[0] is not None:
                tp_pe(inline_tp[0], t)
            filler.step(2 if t < 5 else 1)
            _avT_tile(hb, t, eb)
            if inline_tp[1] is not None:
                tp_pe(inline_tp[1], t)
            filler.step(2 if t < 5 else 1)

    class Filler:
        def __init__(self):
            self.q = deque()

        def add(self, ths, uid=10**9):
            self.q.extend((uid, th) for th in ths)

        def step(self, n=1):
            for _ in range(n):
                if self.q:
                    self.q.popleft()[1]()

        def ensure(self, uid):
            while self.q and self.q[0][0] <= uid:
                self.q.popleft()[1]()

        def drain(self):
            while self.q:
                self.q.popleft()[1]()

    # --- Y -> yT transpose via DMA XBAR (zero-PE thunks for early chunks) ---
    def tpdma_thunks(chs):
        ths = []
        for chh in chs:
            for t in range(NT):
                def run(chh=chh, t=t):
                    nc.sync.dma_start_transpose(
                        yTc[chh][:, t * 128 : (t + 1) * 128],
                        Yn[t][:, chh * 128 : (chh + 1) * 128],
                    )
                ths.append(run)
        return ths

    # --- output projection, split so proj fills the attention tail:
    # proj_partial(t) = co0 contraction k0..4 (needs only chunks 0..4);
    # proj_finish(t) = co0 k5 + full co1 + copies + stores (needs chunk 5,
    # i.e. fin(h10/h11, t) + the inline ch5 transpose).
    pacc = {}

    def proj_partial_thunks(t, ks=range(5)):
        ths = []

        def mk(k, t=t):
            def run():
                if k == 0:
                    pacc[t] = pt([128, 512], "qkv")
                nc.tensor.matmul(
                    pacc[t][:],
                    yTc[k][:, t * 128 : (t + 1) * 128],
                    wpall[:, k * C : k * C + 512],
                    start=(k == 0),
                    stop=False,
                )
            return run

        return [mk(k) for k in ks]

    pacc1 = {}

    def proj_partial1_thunks(t, ks=range(3)):
        def mk(k, t=t):
            def run():
                if k == 0:
                    pacc1[t] = pt([128, 256], "sc")
                nc.tensor.matmul(
                    pacc1[t][:],
                    yTc[k][:, t * 128 : (t + 1) * 128],
                    wpall[:, k * C + 512 : (k + 1) * C],
                    start=(k == 0),
                    stop=False,
                )
            return run

        return [mk(k) for k in ks]

    def proj_finish(t):
        osb = st([128, C], "osb", dtype=F32)
        acc = pacc.pop(t)
        for k in (3, 4, 5):  # k3/k4 here so partials never wait late yT chunks
            nc.tensor.matmul(
                acc[:],
                yTc[k][:, t * 128 : (t + 1) * 128],
                wpall[:, k * C : k * C + 512],
                start=False,
                stop=(k == 5) and bp_d is None,
            )
        if bp_d is not None:
            nc.tensor.matmul(
                acc[:], ones[0:1, 0:128], bp_row[0:1, 0:512],
                start=False, stop=True,
            )
        nc.scalar.copy(osb[:, 0:512], acc[:])
        # store the big half as soon as its copy lands: only the small co1
        # strip transfers after the last compute
        nc.sync.dma_start(y_d.ap()[t * 128 : (t + 1) * 128, 0:512], osb[:, 0:512])
        acc1 = pt([128, 256], "sc")  # sc banks are free once exps are done
        for k in range(NK):
            nc.tensor.matmul(
                acc1[:],
                yTc[k][:, t * 128 : (t + 1) * 128],
                wpall[:, k * C + 512 : (k + 1) * C],
                start=(k == 0),
                stop=(k == NK - 1) and bp_d is None,
            )
        if bp_d is not None:
            nc.tensor.matmul(
                acc1[:], ones[0:1, 0:128], bp_row[0:1, 512:768],
                start=False, stop=True,
            )
        if t == NT - 1:
            nc.vector.tensor_copy(osb[:, 512:768], acc1[:])
        else:
            nc.scalar.copy(osb[:, 512:768], acc1[:])
        nc.sync.dma_start(
            y_d.ap()[t * 128 : (t + 1) * 128, 512:768], osb[:, 512:768]
        )

    # --- schedule ---
    # p-state warmup: the PE needs ~3us of continuous busy to reach max clock;
    # burn identity matmuls into the (otherwise idle) sc bank while the first
    # DMAs are in flight so real matmuls start at peak frequency.
    for i in range(41 if warmup else 0):
        wsc = pt([128, 512], "sc") if i % 4 == 0 else wsc  # noqa: F821
        wid = ident32 if i < 3 else ident  # f32 rows are 4x slower: good early
        nc.tensor.matmul(
            wsc[:, (i % 4) * 128 : (i % 4 + 1) * 128],
            wid[:],
            wid[:],
            start=True,
            stop=True,
        )

    fill = Filler()
    # chunk 0 k-major: all four acc groups (qT0/kT0 x g0/g1) stream each x
    # chunk as it arrives; kT0 accs borrow the (still idle) av banks
    c0_accs = {}
    for k in range(NK):
        for dst, m, gg, tag in (
            (qT[0], 0, 0, "qkv"), (qT[0], 0, 1, "qkv"),
            (kT[0], 6, 0, "av"), (kT[0], 6, 1, "av"),
        ):
            key = (m, gg)
            if k == 0:
                c0_accs[key] = pt([128, 512], tag)
            nc.tensor.matmul(
                c0_accs[key][:],
                wa_of[m][:, k * 128 : (k + 1) * 128],
                xT[k][:, gg * 512 : (gg + 1) * 512],
                start=(k == 0),
                stop=(k == NK - 1),
            )
    for dst, m, gg in (
        (qT[0], 0, 0), (qT[0], 0, 1), (kT[0], 6, 0), (kT[0], 6, 1)
    ):
        dst_ap = dst[:, gg * 512 : (gg + 1) * 512]
        if ba_d is not None:
            nc.vector.tensor_scalar_add(dst_ap, c0_accs[(m, gg)][:], bqk[m][:])
        else:
            nc.scalar.copy(dst_ap, c0_accs[(m, gg)][:])
    emit_sc(0)
    for th in qk_unit_thunks(qT[1], 1):
        th()
    emit_sc(1)
    for th in qk_unit_thunks(kT[1], 7):
        th()
    emit_v(range(0, NT))
    # q/k of same chunk adjacent: qT2,kT2,qT3,kT3,...
    units = [u for c in range(2, NK) for u in ((qT[c], c), (kT[c], c + 6))]
    for h in range(2, H):
        if h >= 4:  # chunk h//2 must be fully emitted before its scores
            fill.ensure(2 * (h // 2 - 2) + 1)
        emit_sc(h)
        if h - 2 < len(units):
            fill.add(qk_unit_thunks(*units[h - 2]), uid=h - 2)
        if h == 10:
            fill.add(tpdma_thunks([0, 1, 2, 3]))
        if h == 11:
            fill.add(tpdma_thunks([4]))
        emit_avT(h - 2, fill)
    # tail: avT(10) covered by proj partials; avT(11) finishes per tile
    fill.add(proj_partial_thunks(0, range(3)))
    fill.add(proj_partial_thunks(1, range(3)))
    emit_avT(10, fill)
    fill.drain()

    def tail_cb(t):
        tp_pe(5, t, tag="av")
        if t >= 1:
            proj_finish(t - 1)
            if t + 1 < NT:  # partials 0,1 are pre-queued before avT(10)
                fill.add(proj_partial_thunks(t + 1, range(3)))
                fill.drain()

    emit_avT(11, fill, cb=tail_cb)
    proj_finish(NT - 1)


# pool buffer counts, patched onto tile_pool via tags at tile() time
_SB_BUFS = {
    "ident": 1, "ident32": 1, "tri": 1, "tri32": 1, "ones": 1, "ones32": 1,
    "bqk": 12, "bv32": 1, "bvrow": 1, "bp32": 1, "bprow": 1,
    "xT": 6, "qkT": 12, "v": 8, "ynat": 8, "yT": 6,
    "wa": 12, "wv": 1, "wp": 1,
    "expt": 5, "rc": 4, "osb": 6,
}
_PS_BUFS = {"qkv": 2, "sc": 3, "av": 3}


_NC_CACHE = {}


def _get_nc(has_battn, has_bproj, num_devices=N_CORES):
    key = (has_battn, has_bproj, num_devices)
    if key not in _NC_CACHE:
        _NC_CACHE[key] = _build_nc(has_battn, has_bproj, num_devices)
    return _NC_CACHE[key]


class _CachedSpmdRunner:
    """Jit the 8-core shard_map once; re-execute on every kernel() call.

    Mirrors concourse.bass2jax.run_bass_via_pjrt's multi-core path, minus the
    per-call retrace/recompile.
    """

    def __init__(self, nc, n_cores):
        import jax
        from jax.sharding import Mesh, PartitionSpec
        from jax.experimental.shard_map import shard_map
        from concourse import bass2jax

        bass2jax.install_neuronx_cc_hook()
        assert nc.dbg_addr is None
        self.n_cores = n_cores
        partition_name = (
            nc.partition_id_tensor.name if nc.partition_id_tensor else None
        )
        in_names, out_names, out_avals = [], [], []
        for alloc in nc.m.functions[0].allocations:
            if not isinstance(alloc, mybir.MemoryLocationSet):
                continue
            name = alloc.memorylocations[0].name
            if alloc.kind == "ExternalInput":
                if name != partition_name:
                    in_names.append(name)
            elif alloc.kind == "ExternalOutput":
                out_names.append(name)
                out_avals.append(
                    jax.core.ShapedArray(
                        tuple(alloc.tensor_shape), mybir.dt.np(alloc.dtype)
                    )
                )
        self.in_names, self.out_names, self.out_avals = (
            in_names, out_names, out_avals,
        )
        all_in_names = list(in_names) + out_names
        if partition_name is not None:
            all_in_names.append(partition_name)

        def _body(*args):
            operands = list(args)
            if partition_name is not None:
                operands.append(bass2jax.partition_id_tensor())
            return tuple(
                bass2jax._bass_exec_p.bind(
                    *operands,
                    out_avals=tuple(out_avals),
                    in_names=tuple(all_in_names),
                    out_names=tuple(out_names),
                    lowering_input_output_aliases=(),
                    sim_require_finite=True,
                    sim_require_nnan=True,
                    nc=nc,
                )
            )

        import numpy as _np

        devices = jax.devices()[:n_cores]
        mesh = Mesh(_np.asarray(devices), ("core",))
        n_io = len(in_names) + len(out_names)
        self.fn = jax.jit(
            shard_map(
                _body,
                mesh=mesh,
                in_specs=(PartitionSpec("core"),) * n_io,
                out_specs=(PartitionSpec("core"),) * len(out_names),
                check_rep=False,
            ),
            keep_unused=True,
        )
        self.sharding = jax.sharding.NamedSharding(mesh, PartitionSpec("core"))

    def run(self, in_maps):
        import jax

        args = []
        for name in self.in_names:
            cat = np.concatenate(
                [np.asarray(m[name]) for m in in_maps], axis=0
            )
            args.append(jax.device_put(cat, self.sharding))
        for av in self.out_avals:
            z = np.zeros((self.n_cores * av.shape[0], *av.shape[1:]), av.dtype)
            args.append(jax.device_put(z, self.sharding))
        outs = self.fn(*args)
        return [
            {
                name: np.asarray(outs[i]).reshape(
                    self.n_cores, *self.out_avals[i].shape
                )[c]
                for i, name in enumerate(self.out_names)
            }
            for c in range(self.n_cores)
        ]


_RUNNER_CACHE = {}


def _bf16(a):
    import ml_dtypes

    return np.ascontiguousarray(np.asarray(a).astype(ml_dtypes.bfloat16))


def kernel(x, W_attn, b_attn, W_proj, b_proj):
    x = _bf16(np.asarray(x).transpose(0, 2, 1))  # [B, C, T]: device wants x^T
    W_attn = _bf16(W_attn)
    W_proj = _bf16(W_proj)
    b_attn = np.asarray(b_attn, dtype=np.float32)
    b_proj = np.asarray(b_proj, dtype=np.float32)
    has_battn = bool(np.any(b_attn != 0.0))
    has_bproj = bool(np.any(b_proj != 0.0))

    nc = _get_nc(has_battn, has_bproj)
    in_maps = []
    for b in range(N_CORES):
        m = {"x": x[b], "W_attn": W_attn, "W_proj": W_proj}
        if has_battn:
            m["b_attn"] = b_attn
        if has_bproj:
            m["b_proj"] = b_proj
        in_maps.append(m)

    key = (has_battn, has_bproj)
    try:
        if key not in _RUNNER_CACHE:
            _RUNNER_CACHE[key] = _CachedSpmdRunner(nc, N_CORES)
        results = _RUNNER_CACHE[key].run(in_maps)
    except Exception:
        results = run_bass_kernel_spmd(
            nc, in_maps, core_ids=list(range(N_CORES))
        ).results
    return np.stack([results[b]["y"] for b in range(N_CORES)]).astype(np.float32)



# revision 91
# speedup vs baseline: 1.1344x; 1.1344x over previous
"""Trainium2 Bass kernel for causal multi-head attention block.

Module: qkv = x @ W_attn + b_attn; causal softmax((q k^T)/sqrt(C)); y = (attn @ v) @ W_proj + b_proj
Shapes (hardcoded): x [8, 1024, 768], W_attn [768, 2304], W_proj [768, 768], H=12, D=64.

Sharding: data parallel over batch - core b computes batch element b (B == n_cores == 8).
No collectives; host scatters x and gathers y.

QKV + its 768-deep contraction run in fp8e4 DoubleRow (2 fp8 weights/PE cell,
K=256 per matmul, 0.5 cyc/out-col): x and W_attn are split hi/lo on the host
(x ~ hi + lo with lo = fp8(x - hi)) and the product takes three fp8 terms
  x@W ~ x_hi@W_hi + x_lo@W_hi + x_hi@W_lo
(4.5 cyc/out-col vs bf16's 6) at ~bf16 accuracy. W_attn is pre-scaled by
WS=16 for fp8 range; the q*k scale folds into the exp scale and the V scale
into W_proj (1/WS, bf16). Host pre-packs DoubleRow pair layouts:
  x_hi/x_lo [384, 2048]: pair p rows hold x^T chunks 2p | 2p+1 side by side
  wqk [768, 3072]: chunk-pair-major m order (0,6),(1,7).., [hi|lo] blocks
  wv  [768, 1536]: V columns, hi | lo

Per-core dataflow (scores/AV matmuls bf16, fp32 PSUM accumulate):
  1. paired loads (one ~625ns HWDGE descriptor each): wa(0,6), the three
     x_hi pairs (all Q,K needs), wa(1,7), x_lo pairs, wv, wa rest, wpall
  2. QT,KT [c,1024] = (W chunk).T @ x_hi  (6 DoubleRow matmuls per 512-col
     group; V keeps all 9 terms)
     V natural [tok, 12*(64+1)]: per head [V_h | 1.0] so the AV matmul emits
     the softmax denominator as the 65th output column.
  3. per head h, blocks (query-group g, key-tile j) packed first-fit-decreasing
     into exactly-full 512-word PSUM bins; two bins share a 2-bank PSUM slot
     so one 1024-wide Exp amortizes the ScalarE access overhead:
       scoresT[tk,tq] = KT_h.T @ QT_h   (causal-tight tiles only)
       e = Exp(scoresT*scale)           (ScalarE, PSUM->SBUF bf16)
       diag tiles *= upper-tri mask     (Pool)
       avT[tq, 65] += e_block.T @ [V_h|1]
       Y[t, h*64:] = avT[:,0:64] * recip(avT[:,64])  (DVE)
     Scores run two heads ahead of AV; QK-chunk matmuls interleave as PE
     filler so exp latency and fin pacing never stall the PE.
  4. Y -> yT: one DMA-XBAR descriptor per token tile transposes chunks 0..3
     (after head 7), chunk 4 follows head 9, chunk 5 on the PE in the tail.
     Heads 10/11 AV interleave per tile with the per-tile proj finishes;
     proj partials (k0..3) run as filler. One bf16 store per token tile.
"""

import math
from collections import deque

import numpy as np

import concourse.bass as bass
import concourse.mybir as mybir
from concourse import bacc, tile
from concourse.bass_utils import run_bass_kernel_spmd
from concourse.masks import make_identity, make_upper_triangular

N_CORES = 8
B, T, C = 8, 1024, 768
H, D = 12, 64
C3 = 3 * C
NT = T // 128          # 8 token tiles
NK = C // 128          # 6 contraction chunks
NP = NK // 2           # 3 DoubleRow contraction pairs
NG = T // 512          # 2 query groups of 512
WS = 16.0              # fp8 pre-scale on W_attn
SCALE = 1.0 / math.sqrt(C)
ESCALE = SCALE / (WS * WS)   # q,k both carry WS

F32 = mybir.dt.float32
BF16 = mybir.dt.bfloat16
F8 = mybir.dt.float8e4
DR = mybir.MatmulPerfMode.DoubleRow
MULT = mybir.AluOpType.mult
EXP = mybir.ActivationFunctionType.Exp

# fp8 term order per contraction pair: (x_lo?, w_lo?); x_lo last so the
# matmuls stream as the hi/lo DMAs land. Q,K drop the x_lo correction:
# softmax damps the resulting ~0.01 absolute score error (end-to-end rel
# err ~1.2e-2 vs the 2e-2 gate) and it saves 3 of 9 matmuls per chunk.
TERMS = ((0, 0), (0, 1), (1, 0))
QK_TERMS = ((0, 0), (0, 1))


def _head_plan():
    """Causal score blocks for one head, packed into PSUM tiles.

    Returns (tiles, ecol): tiles is a list of (blocks, tile_width) where each
    block is (g, j, nw, tq0, diag, sc_off, e_off); ecol[(t, j)] gives the
    column in the head's e tile holding scoresT[:, tq-tile t] for key-tile j.
    """
    blocks = []
    for g in range(NG):
        for j in range(4 * g + 4):
            nw = 512 - max(j * 128 - g * 512, 0)
            tq0 = g * 512 + (512 - nw)
            blocks.append([g, j, nw, tq0])
    # first-fit-decreasing into 512-wide bins (bins end up exactly full)
    blocks.sort(key=lambda b: -b[2])
    bins = []
    for b in blocks:
        for bn in bins:
            if sum(x[2] for x in bn) + b[2] <= 512:
                bn.append(b)
                break
        else:
            bins.append([b])
    # one bin per [128,512] PSUM tile; assign sc/e offsets
    tiles = []
    e_base = 0
    for bn in bins:
        tb, off = [], 0
        for g, j, nw, tq0 in bn:
            diag = j * 128 == tq0
            tb.append((g, j, nw, tq0, diag, off, e_base + off))
            off += nw
        tiles.append((tb, off))
        e_base += off
    assert e_base == 4608
    ecol = {}
    for tb, _ in tiles:
        for g, j, nw, tq0, diag, sc_off, e_off in tb:
            for t in range(4 * g, 4 * g + 4):
                lo = t * 128
                if tq0 <= lo and lo + 128 <= tq0 + nw:
                    ecol[(t, j)] = e_off + lo - tq0
    for t in range(NT):
        for j in range(t + 1):
            assert (t, j) in ecol, (t, j)
    return tiles, ecol


_PLAN_TILES, _PLAN_ECOL = _head_plan()


def _build_nc(has_battn: bool, has_bproj: bool, num_devices: int, repeat: int = 1):
    nc = bacc.Bacc(
        "TRN2", target_bir_lowering=False, debug=False, num_devices=num_devices
    )

    xh_d = nc.dram_tensor("x_hi", (NP * 128, 2 * T), F8, kind="ExternalInput")
    xl_d = nc.dram_tensor("x_lo", (NP * 128, 2 * T), F8, kind="ExternalInput")
    wqk_d = nc.dram_tensor("wqk", (C, 2 * 12 * 128), F8, kind="ExternalInput")
    wv_d = nc.dram_tensor("wv", (C, 2 * C), F8, kind="ExternalInput")
    wp_d = nc.dram_tensor("W_proj", (C, C), BF16, kind="ExternalInput")
    ba_d = bp_d = None
    if has_battn:
        ba_d = nc.dram_tensor("b_attn", (C3,), F32, kind="ExternalInput")
    if has_bproj:
        bp_d = nc.dram_tensor("b_proj", (C,), F32, kind="ExternalInput")
    y_d = nc.dram_tensor("y", (T, C), BF16, kind="ExternalOutput")

    with tile.TileContext(nc) as tc:
        with (
            tc.tile_pool(name="sb", bufs=1) as sb,
            tc.tile_pool(name="ps", bufs=1, space="PSUM") as ps,
        ):
            for r in range(repeat):
                _emit(nc, sb, ps, xh_d, xl_d, wqk_d, wv_d, wp_d, ba_d, bp_d, y_d,
                      warmup=(r == 0))
    nc.compile()
    return nc


def _emit(nc, sb, ps, xh_d, xl_d, wqk_d, wv_d, wp_d, ba_d, bp_d, y_d, warmup=True):
    def st(shape, tag, dtype=BF16, name=None):
        return sb.tile(list(shape), dtype, tag=tag, bufs=_SB_BUFS[tag],
                       name=name or tag)

    def pt(shape, tag, dtype=F32, name=None):
        return ps.tile(list(shape), dtype, tag=tag, bufs=_PS_BUFS[tag],
                       name=name or tag)

    # p-state warmup: the PE needs ~3us of continuous busy to reach max clock;
    # burn matmuls on a memset scratch into the (otherwise idle) sc bank while
    # the first DMAs are in flight so real matmuls start at peak frequency.
    # Emitted before the identity/mask builds so the PE starts ~1.2us sooner.
    if warmup:
        wu32 = st([128, 128], "wu32", dtype=F32)
        nc.gpsimd.memset(wu32[:], 0.0)  # Pool runs right after the preamble
        wu = st([128, 128], "wu")
        nc.vector.memset(wu[:], 0.0)
        for i in range(20):
            wsc = pt([128, 1024], "sc") if i % 8 == 0 else wsc  # noqa: F821
            wid = wu32 if i < 3 else wu  # f32 rows are 4x slower: good early
            nc.tensor.matmul(
                wsc[:, (i % 8) * 128 : (i % 8 + 1) * 128],
                wid[:],
                wid[:],
                start=True,
                stop=True,
            )

    # --- constants ---
    ident32 = st([128, 128], "ident32", dtype=F32)
    make_identity(nc, ident32[:])
    ident = st([128, 128], "ident")
    nc.vector.tensor_copy(ident[:], ident32[:])
    tri32 = st([128, 128], "tri32", dtype=F32)
    make_upper_triangular(nc, tri32[:], val=1.0, diag=True)  # tri[r,c]=1 iff c>=r
    tri = st([128, 128], "tri")
    nc.vector.tensor_copy(tri[:], tri32[:])

    bqk = []
    ones = bv_row = bp_row = None
    if ba_d is not None or bp_d is not None:
        ones32 = st([1, 128], "ones32", dtype=F32)
        nc.gpsimd.memset(ones32[:], 1.0)
        ones = st([1, 128], "ones")
        nc.vector.tensor_copy(ones[:], ones32[:])
    if ba_d is not None:
        for m in range(12):  # Q,K outchan tiles 0..1535
            bt = st([128, 1], "bqk", dtype=F32)
            nc.sync.dma_start(bt[:], ba_d.ap()[m * 128 : (m + 1) * 128])
            bqk.append(bt)
        bv32 = st([1, C], "bv32", dtype=F32)
        nc.sync.dma_start(bv32[:], ba_d.ap()[2 * C : 3 * C])
        bv_row = st([1, C], "bvrow")
        nc.vector.tensor_copy(bv_row[:], bv32[:])
    if bp_d is not None:
        bp32 = st([1, C], "bp32", dtype=F32)
        nc.sync.dma_start(bp32[:], bp_d.ap())
        bp_row = st([1, C], "bprow")
        nc.vector.tensor_copy(bp_row[:], bp32[:])

    # --- persistent activations ---
    qT = [st([128, T], "qkT", name=f"qT{c}") for c in range(NK)]
    kT = [st([128, T], "qkT", name=f"kT{c}") for c in range(NK)]
    vt = [st([128, H * 65], "v", name=f"v{t}") for t in range(NT)]
    Yn = [st([128, C], "ynat", name=f"Y{t}") for t in range(NT)]
    yTall = st([128, NK * T], "yT")  # chunk-major y^T; one tensor so a single
    #                                  DMA descriptor can transpose 5 chunks

    def yT_ap(k, c0, c1):
        return yTall[:, k * T + c0 : k * T + c1]


    # --- DMAs (SP queue; emission order = issue order; each descriptor
    # costs ~625ns HWDGE + ~650ns DGE delay, so loads are batched) ---
    was = []

    def load_wa2(c):
        # [128, 6, 512]: per k-chunk, q-tile m=c | k-tile m=c+6, each hi|lo
        wam = st([128, NK * 512], "wa", dtype=F8, name=f"wa{c}")
        nc.sync.dma_start(
            wam[:],
            wqk_d.ap()[:, c * 512 : (c + 1) * 512].rearrange(
                "(k p) n -> p k n", p=128
            ),
        )
        return wam

    xF = []                     # pair p: [128, hi 2T | lo 2T]

    def load_x(p, lo):
        if not lo:
            xt = st([128, 4 * T], "xT", dtype=F8, name=f"x{p}")
            xF.append(xt)
            nc.sync.dma_start(
                xt[:, 0 : 2 * T], xh_d.ap()[p * 128 : (p + 1) * 128, :]
            )
        else:
            nc.sync.dma_start(
                xF[p][:, 2 * T :], xl_d.ap()[p * 128 : (p + 1) * 128, :]
            )

    # Q,K contract only x_hi, so all hi pairs load first; the lo pairs are
    # needed by V only and follow with wv
    was.append(load_wa2(0))     # Q,K chunk 0
    for p in range(NP):
        load_x(p, lo=False)
    was.append(load_wa2(1))
    for p in range(NP):
        load_x(p, lo=True)
    # all V columns, every k-chunk, hi then lo, in one 3-D-AP DMA
    wvall = st([128, 2 * NK * C], "wv", dtype=F8)
    nc.sync.dma_start(
        wvall[:],
        wv_d.ap().rearrange("(k p) n -> p k n", p=128),
    )
    for c in range(2, NK):
        was.append(load_wa2(c))
    wpall = st([128, NK * C], "wp")
    nc.sync.dma_start(
        wpall[:], wp_d.ap().rearrange("(k p) n -> p k n", p=128)
    )

    # --- DoubleRow operand views (all 3-D APs [K, 2, n]: dim 1 picks the
    # two contraction chunks of pair p) ---
    def wa_ap(m, p, wlo):
        # lhsT [128, 2, 128]: pair p weights, hi or lo column block
        v = was[m % 6][:].rearrange("q (k n) -> q k n", n=512)
        off = (m // 6) * 256 + wlo * 128
        return v[:, 2 * p : 2 * p + 2, off : off + 128]

    def x_ap(p, xlo, c0, c1):
        # rhs [128, 2, c1-c0]
        return xF[p][:, xlo * 2 * T : (xlo + 1) * 2 * T].rearrange(
            "q (two t) -> q two t", two=2
        )[:, :, c0:c1]

    def wv_ap(p, wlo, co, w):
        # rhs [128, 2, w]: V out-columns co:co+w of pair p, hi or lo
        v = wvall[:].rearrange("q (k n) -> q k n", n=2 * C)
        return v[:, 2 * p : 2 * p + 2, wlo * C + co : wlo * C + co + w]

    # --- QK-chunk unit: thunk list (filler granularity = 1 matmul) ---
    def qk_unit_thunks(dst, m, cp_eng=None):
        terms = QK_TERMS
        ths = []
        for g in range(NG):
            box = {}

            def mk(i, p, xlo, wlo, g=g, box=box):
                def run():
                    if i == 0:
                        box["acc"] = pt([128, 512], "qkv")
                    nc.tensor.matmul(
                        box["acc"][:],
                        wa_ap(m, p, wlo),
                        x_ap(p, xlo, g * 512, (g + 1) * 512),
                        start=(i == 0),
                        stop=(i == len(terms) * NP - 1),
                        perf_mode=DR,
                    )
                return run

            i = 0
            for p in range(NP):
                for xlo, wlo in terms:
                    ths.append(mk(i, p, xlo, wlo))
                    i += 1

            def cp(g=g, box=box, m=m):
                dst_ap = dst[:, g * 512 : (g + 1) * 512]
                if ba_d is not None:
                    nc.vector.tensor_scalar_add(dst_ap, box["acc"][:], bqk[m][:])
                elif cp_eng == "act":
                    nc.scalar.copy(dst_ap, box["acc"][:])
                else:
                    nc.vector.tensor_copy(dst_ap, box["acc"][:])

            ths.append(cp)
        return ths

    # --- V natural [tok, 12*(64+1)] ---
    def emit_v(trange):
        for t in trange:
            nc.gpsimd.memset(vt[t][:], 1.0)  # ones cols; data cols overwritten
            for co, w in ((0, 512), (512, 256)):
                acc = pt([128, w], "qkv")
                i = 0
                for p in range(NP):
                    for xlo, wlo in TERMS:
                        nc.tensor.matmul(
                            acc[:],
                            x_ap(p, xlo, t * 128, (t + 1) * 128),
                            wv_ap(p, wlo, co, w),
                            start=(i == 0),
                            stop=(i == 3 * NP - 1) and ba_d is None,
                            perf_mode=DR,
                        )
                        i += 1
                if ba_d is not None:
                    nc.tensor.matmul(
                        acc[:],
                        ones[0:1, 0:128],
                        bv_row[0:1, co : co + w],
                        start=False,
                        stop=True,
                    )
                nc.vector.tensor_copy(
                    vt[t][:].rearrange("p (h e) -> p h e", e=65)[
                        :, co // D : (co + w) // D, 0:D
                    ],
                    acc[:].rearrange("p (h e) -> p h e", e=D),
                )

    # --- per-head scores + exp + mask ---
    e_of = {}

    def emit_sc(h, filler=None):
        ch, off = h // 2, (h % 2) * 64
        e = st([128, 4608], "expt", name=f"e{h}")
        e_of[h] = e
        # two 512-wide plan tiles share one 2-bank PSUM slot; a single
        # 1024-wide exp halves the per-instruction Activation overhead
        for i in range(0, len(_PLAN_TILES), 2):
            if filler is not None and i:
                filler.step(1)
            pair = _PLAN_TILES[i : i + 2]
            sc = pt([128, 1024], "sc")
            width = 0
            for half, (tb, tw) in enumerate(pair):
                for g, j, nw, tq0, diag, sc_off, e_off in tb:
                    nc.tensor.matmul(
                        sc[:, half * 512 + sc_off : half * 512 + sc_off + nw],
                        kT[ch][off : off + 64, j * 128 : (j + 1) * 128],
                        qT[ch][off : off + 64, tq0 : tq0 + nw],
                        start=True,
                        stop=True,
                    )
                width = half * 512 + tw
            e_base = pair[0][0][0][6]
            nc.scalar.activation(
                e[:, e_base : e_base + width], sc[:, 0:width], EXP, scale=ESCALE
            )
            for tb, tw in pair:
                for g, j, nw, tq0, diag, sc_off, e_off in tb:
                    if diag:
                        nc.gpsimd.tensor_tensor(
                            e[:, e_off : e_off + 128],
                            e[:, e_off : e_off + 128],
                            tri[:],
                            op=MULT,
                        )

    # --- per-head AV^T + fused-normalization fin, interleaved with filler ---
    def tp_pe(chh, t, tag="qkv"):
        ptt = pt([128, 128], tag, dtype=BF16)
        nc.tensor.transpose(
            ptt[:], Yn[t][:, chh * 128 : (chh + 1) * 128], ident[:]
        )
        nc.vector.tensor_copy(yT_ap(chh, t * 128, (t + 1) * 128), ptt[:])

    def _avT_tile(h, t, e):
        av = pt([128, 65], "av")
        for j in range(t + 1):
            nc.tensor.matmul(
                av[:],
                e[:, _PLAN_ECOL[(t, j)] : _PLAN_ECOL[(t, j)] + 128],
                vt[j][:, h * 65 : (h + 1) * 65],
                start=(j == 0),
                stop=(j == t),
            )
        rc = st([128, 1], "rc", dtype=F32)
        nc.vector.reciprocal(rc[:], av[:, 64:65])
        nc.vector.tensor_scalar_mul(
            Yn[t][:, h * D : (h + 1) * D], av[:, 0:D], rc[:]
        )

    def emit_avT(h, filler, inline_tp=None, cb=None, order=None):
        e = e_of.pop(h)
        for t in (range(NT) if order is None else order):
            _avT_tile(h, t, e)
            if inline_tp is not None:
                tp_pe(inline_tp, t)
            if cb is not None:
                cb(t)
            filler.step(2 if t % 2 == 0 else 3)

    class Filler:
        def __init__(self):
            self.q = deque()

        def add(self, ths, uid=10**9):
            self.q.extend((uid, th) for th in ths)

        def step(self, n=1):
            for _ in range(n):
                if self.q:
                    self.q.popleft()[1]()

        def ensure(self, uid):
            while self.q and self.q[0][0] <= uid:
                self.q.popleft()[1]()

        def drain(self):
            while self.q:
                self.q.popleft()[1]()

    # --- Y -> yT transpose via DMA XBAR: one descriptor per token tile
    # covers chunks 0..3 (emitted right after head 7's fin, so the proj
    # partials can fill the exp-paced avT(9)/avT(10) stretches); chunk 4
    # follows head 9's fin ---
    def tpdma03(t):
        nc.sync.dma_start_transpose(
            yTall[:].rearrange("q (ch t) -> q ch t", t=T)[
                :, 0:4, t * 128 : (t + 1) * 128
            ],
            Yn[t][:, 0:512],
        )

    def tpdma4(t):
        nc.sync.dma_start_transpose(
            yT_ap(4, t * 128, (t + 1) * 128), Yn[t][:, 512:640]
        )

    # --- output projection, split so proj fills the attention tail:
    # proj_partial(t) = co0 contraction k0..4 (needs only chunks 0..4);
    # proj_finish(t) = co0 k5 + full co1 + copies + stores (needs chunk 5,
    # i.e. fin(h10/h11, t) + the inline ch5 transpose).
    pacc = {}

    def proj_partial_thunks(t, ks=range(5), stop_last=False):
        ks = list(ks)

        def mk(k, t=t):
            def run():
                if k == 0:
                    pacc[t] = pt([128, 512], "qkv")
                nc.tensor.matmul(
                    pacc[t][:],
                    yT_ap(k, t * 128, (t + 1) * 128),
                    wpall[:, k * C : k * C + 512],
                    start=(k == 0),
                    stop=(k == ks[-1]) and stop_last,
                )
            return run

        return [mk(k) for k in ks]

    pacc1 = {}

    def proj_partial1_thunks(t, ks=range(3), stop_last=False):
        ks = list(ks)

        def mk(k, t=t):
            def run():
                if k == 0:
                    pacc1[t] = pt([128, 256], "sc")
                nc.tensor.matmul(
                    pacc1[t][:],
                    yT_ap(k, t * 128, (t + 1) * 128),
                    wpall[:, k * C + 512 : (k + 1) * C],
                    start=(k == 0),
                    stop=(k == ks[-1]) and stop_last,
                )
            return run

        return [mk(k) for k in ks]

    def proj_finish_b(t):
        # everything that does NOT need the just-transposed chunk 5
        # co0 k4 (partials covered k0..3) + co1 k0..4
        nc.tensor.matmul(
            pacc[t][:],
            yT_ap(4, t * 128, (t + 1) * 128),
            wpall[:, 4 * C : 4 * C + 512],
            start=False,
            stop=False,
        )
        if t not in pacc1:  # tiles 0,1 are pre-filled as avT(10) filler
            pacc1[t] = pt([128, 256], "sc")  # sc banks free once exps done
            for k in range(5):
                nc.tensor.matmul(
                    pacc1[t][:],
                    yT_ap(k, t * 128, (t + 1) * 128),
                    wpall[:, k * C + 512 : (k + 1) * C],
                    start=(k == 0),
                    stop=False,
                )

    def proj_finish_a(t):
        # k5 terms + copies + the single store descriptor (the HWDGE queue
        # costs ~625ns per descriptor, so one wide store beats split ones)
        osb = st([128, C], "osb")  # bf16: halves the store transfer time
        acc = pacc.pop(t)
        acc1 = pacc1.pop(t)
        nc.tensor.matmul(
            acc[:],
            yT_ap(5, t * 128, (t + 1) * 128),
            wpall[:, 5 * C : 5 * C + 512],
            start=False,
            stop=bp_d is None,
        )
        if bp_d is not None:
            nc.tensor.matmul(
                acc[:], ones[0:1, 0:128], bp_row[0:1, 0:512],
                start=False, stop=True,
            )
        nc.scalar.copy(osb[:, 0:512], acc[:])
        nc.tensor.matmul(
            acc1[:],
            yT_ap(5, t * 128, (t + 1) * 128),
            wpall[:, 5 * C + 512 : 6 * C],
            start=False,
            stop=bp_d is None,
        )
        if bp_d is not None:
            nc.tensor.matmul(
                acc1[:], ones[0:1, 0:128], bp_row[0:1, 512:768],
                start=False, stop=True,
            )
        nc.vector.tensor_copy(osb[:, 512:768], acc1[:])
        nc.sync.dma_start(y_d.ap()[t * 128 : (t + 1) * 128, :], osb[:])

    # --- schedule ---
    fill = Filler()
    # chunk 0 term-major: all four acc groups (qT0/kT0 x g0/g1) stream each
    # x pair as it arrives; kT0 accs borrow the (still idle) av banks
    c0_accs = {}
    c0_i = {}
    for p in range(NP):
        for xlo, wlo in TERMS:
            for dst, m, gg, tag in (
                (qT[0], 0, 0, "qkv"), (qT[0], 0, 1, "qkv"),
                (kT[0], 6, 0, "av"), (kT[0], 6, 1, "av"),
            ):
                terms = QK_TERMS
                if (xlo, wlo) not in terms:
                    continue
                key = (m, gg)
                j = c0_i.get(key, 0)
                if j == 0:
                    c0_accs[key] = pt([128, 512], tag)
                nc.tensor.matmul(
                    c0_accs[key][:],
                    wa_ap(m, p, wlo),
                    x_ap(p, xlo, gg * 512, (gg + 1) * 512),
                    start=(j == 0),
                    stop=(j == len(terms) * NP - 1),
                    perf_mode=DR,
                )
                c0_i[key] = j + 1
    for dst, m, gg in (
        (qT[0], 0, 0), (qT[0], 0, 1), (kT[0], 6, 0), (kT[0], 6, 1)
    ):
        dst_ap = dst[:, gg * 512 : (gg + 1) * 512]
        if ba_d is not None:
            nc.vector.tensor_scalar_add(dst_ap, c0_accs[(m, gg)][:], bqk[m][:])
        else:
            nc.scalar.copy(dst_ap, c0_accs[(m, gg)][:])
    emit_sc(0)
    for th in qk_unit_thunks(qT[1], 1):
        th()
    emit_sc(1)
    for th in qk_unit_thunks(kT[1], 7):
        th()
    emit_v(range(0, NT))
    # q/k of same chunk adjacent: qT2,kT2,qT3,kT3,...
    units = [u for c in range(2, NK) for u in ((qT[c], c), (kT[c], c + 6))]
    for h in range(2, H):
        if h >= 4:  # chunk h//2 must be fully emitted before its scores
            fill.ensure(2 * (h // 2 - 2) + 1)
        emit_sc(h, fill)
        if h - 2 < len(units):
            fill.add(qk_unit_thunks(*units[h - 2]), uid=h - 2)
        if h == 10:
            # the QK filler units are exhausted; proj partials for the first
            # two tiles (their transposes landed during avT(7)) feed the
            # exp-paced avT(8)/avT(9) windows and park until the tail
            fill.add(proj_partial_thunks(0, range(4)))
            fill.add(proj_partial_thunks(1, range(4)))
        cb = {9: tpdma03, 11: tpdma4}.get(h)
        emit_avT(h - 2, fill, cb=cb)
    # tail: the last two heads interleave per tile so the ch5 transpose and
    # the proj finishes chase each tile's fins directly; tile 0 is processed
    # last so only 2 AV matmuls sit on the final dependency chain
    fill.drain()
    e10, e11 = e_of.pop(10), e_of.pop(11)
    for t in range(NT):
        _avT_tile(10, t, e10)
        _avT_tile(11, t, e11)
        if t >= 1:
            proj_finish_a(t - 1)
            if t + 1 < NT:  # partials(t+1) reuse the slot finish_a just freed
                for th in proj_partial_thunks(t + 1, range(4)):
                    th()
        tp_pe(5, t, tag="av")
        proj_finish_b(t)
    proj_finish_a(NT - 1)


# pool buffer counts, patched onto tile_pool via tags at tile() time
_SB_BUFS = {
    "ident": 1, "ident32": 1, "tri": 1, "tri32": 1, "ones": 1, "ones32": 1,
    "bqk": 12, "bv32": 1, "bvrow": 1, "bp32": 1, "bprow": 1,
    "xT": 3, "qkT": 12, "v": 8, "ynat": 8, "yT": 1, "wu": 1, "wu32": 1,
    "wa": 6, "wv": 1, "wp": 1,
    "expt": 5, "rc": 4, "osb": 6, 
}
_PS_BUFS = {"qkv": 2, "sc": 2, "av": 2}


_NC_CACHE = {}


def _get_nc(has_battn, has_bproj, num_devices=N_CORES):
    key = (has_battn, has_bproj, num_devices)
    if key not in _NC_CACHE:
        _NC_CACHE[key] = _build_nc(has_battn, has_bproj, num_devices)
    return _NC_CACHE[key]


class _CachedSpmdRunner:
    """Jit the 8-core shard_map once; re-execute on every kernel() call.

    Mirrors concourse.bass2jax.run_bass_via_pjrt's multi-core path, minus the
    per-call retrace/recompile.
    """

    def __init__(self, nc, n_cores):
        import jax
        from jax.sharding import Mesh, PartitionSpec
        from jax.experimental.shard_map import shard_map
        from concourse import bass2jax

        bass2jax.install_neuronx_cc_hook()
        assert nc.dbg_addr is None
        self.n_cores = n_cores
        partition_name = (
            nc.partition_id_tensor.name if nc.partition_id_tensor else None
        )
        in_names, out_names, out_avals = [], [], []
        for alloc in nc.m.functions[0].allocations:
            if not isinstance(alloc, mybir.MemoryLocationSet):
                continue
            name = alloc.memorylocations[0].name
            if alloc.kind == "ExternalInput":
                if name != partition_name:
                    in_names.append(name)
            elif alloc.kind == "ExternalOutput":
                out_names.append(name)
                out_avals.append(
                    jax.core.ShapedArray(
                        tuple(alloc.tensor_shape), mybir.dt.np(alloc.dtype)
                    )
                )
        self.in_names, self.out_names, self.out_avals = (
            in_names, out_names, out_avals,
        )
        all_in_names = list(in_names) + out_names
        if partition_name is not None:
            all_in_names.append(partition_name)

        def _body(*args):
            operands = list(args)
            if partition_name is not None:
                operands.append(bass2jax.partition_id_tensor())
            return tuple(
                bass2jax._bass_exec_p.bind(
                    *operands,
                    out_avals=tuple(out_avals),
                    in_names=tuple(all_in_names),
                    out_names=tuple(out_names),
                    lowering_input_output_aliases=(),
                    sim_require_finite=True,
                    sim_require_nnan=True,
                    nc=nc,
                )
            )

        import numpy as _np

        devices = jax.devices()[:n_cores]
        mesh = Mesh(_np.asarray(devices), ("core",))
        n_io = len(in_names) + len(out_names)
        self.fn = jax.jit(
            shard_map(
                _body,
                mesh=mesh,
                in_specs=(PartitionSpec("core"),) * n_io,
                out_specs=(PartitionSpec("core"),) * len(out_names),
                check_rep=False,
            ),
            keep_unused=True,
        )
        self.sharding = jax.sharding.NamedSharding(mesh, PartitionSpec("core"))

    def run(self, in_maps):
        import jax

        args = []
        for name in self.in_names:
            cat = np.concatenate(
                [np.asarray(m[name]) for m in in_maps], axis=0
            )
            args.append(jax.device_put(cat, self.sharding))
        for av in self.out_avals:
            z = np.zeros((self.n_cores * av.shape[0], *av.shape[1:]), av.dtype)
            args.append(jax.device_put(z, self.sharding))
        outs = self.fn(*args)
        return [
            {
                name: np.asarray(outs[i]).reshape(
                    self.n_cores, *self.out_avals[i].shape
                )[c]
                for i, name in enumerate(self.out_names)
            }
            for c in range(self.n_cores)
        ]


_RUNNER_CACHE = {}


def _bf16(a):
    import ml_dtypes

    return np.ascontiguousarray(np.asarray(a).astype(ml_dtypes.bfloat16))


def _split8(a):
    import ml_dtypes

    a = np.asarray(a, dtype=np.float32)
    hi = a.astype(ml_dtypes.float8_e4m3)
    lo = (a - hi.astype(np.float32)).astype(ml_dtypes.float8_e4m3)
    return hi, lo


def _pair_x(xT):
    """[C, T] -> [NP*128, 2T]: pair p rows hold chunks 2p | 2p+1 side by side."""
    return np.ascontiguousarray(
        xT.reshape(NP, 2, 128, T).transpose(0, 2, 1, 3).reshape(NP * 128, 2 * T)
    )


def _prep_w(W_attn, W_proj):
    """Host-side fp8 split + DoubleRow pack of the weights."""
    wh, wl = _split8(np.asarray(W_attn, np.float32) * WS)
    qk_h = np.asarray(wh[:, : 2 * C]).reshape(C, 12, 1, 128)
    qk_l = np.asarray(wl[:, : 2 * C]).reshape(C, 12, 1, 128)
    both = np.concatenate([qk_h, qk_l], axis=2)  # [C, m, hi/lo, 128]
    perm = [m for c in range(NK) for m in (c, c + 6)]  # q/k chunk pairs
    wqk = np.ascontiguousarray(both[:, perm].reshape(C, 2 * 12 * 128))
    wv = np.ascontiguousarray(
        np.concatenate([wh[:, 2 * C :], wl[:, 2 * C :]], axis=1)
    )
    wp = _bf16(np.asarray(W_proj, np.float32) / WS)
    return {"wqk": wqk, "wv": wv, "W_proj": wp}


def _prep_x(xb):
    """One batch element [T, C] -> {'x8'}: pair rows, [hi 2T | lo 2T]."""
    xT = np.ascontiguousarray(np.asarray(xb, np.float32).T)
    hi, lo = _split8(xT)
    return {"x_hi": _pair_x(hi), "x_lo": _pair_x(lo)}


def kernel(x, W_attn, b_attn, W_proj, b_proj):
    b_attn = np.asarray(b_attn, dtype=np.float32)
    b_proj = np.asarray(b_proj, dtype=np.float32)
    has_battn = bool(np.any(b_attn != 0.0))
    has_bproj = bool(np.any(b_proj != 0.0))

    shared = _prep_w(W_attn, W_proj)
    nc = _get_nc(has_battn, has_bproj)
    in_maps = []
    for b in range(N_CORES):
        m = dict(shared)
        m.update(_prep_x(np.asarray(x)[b]))
        if has_battn:
            m["b_attn"] = b_attn * WS
        if has_bproj:
            m["b_proj"] = b_proj
        in_maps.append(m)

    key = (has_battn, has_bproj)
    try:
        if key not in _RUNNER_CACHE:
            _RUNNER_CACHE[key] = _CachedSpmdRunner(nc, N_CORES)
        results = _RUNNER_CACHE[key].run(in_maps)
    except Exception:
        results = run_bass_kernel_spmd(
            nc, in_maps, core_ids=list(range(N_CORES))
        ).results
    return np.stack([results[b]["y"] for b in range(N_CORES)]).astype(np.float32)
